# revision 2
# baseline (speedup 1.0000x reference)
"""AlexNet forward pass on 8 Trainium2 NeuronCores.

Strategy: pure data parallel over batch for the conv stack (16 images
per core, conv weights replicated), tensor parallel for the FC layers
(activations all-gathered, each core computes a 1/8 column slice of
FC1/FC2/FC3). Convs are shift-and-matmul over kernel offsets with
channels on the partition dim; matmuls and activations run in bf16
(PSUM accumulation in fp32), LRN channel-window sums run on the PE via
banded matrices and the d^-3/4 power via fused Ln/Exp on the scalar
engine (one combined ln+exp table set, loaded once). Conv1 packs the
full 11x11 kernel into the contraction dim (K=122 incl. bias row, one
matmul per input channel). FC layers run feature-major (weights as
lhsT, activations as rhs) so every DRAM store/load is contiguous.

kernel(**inputs) takes the full unsharded inputs and returns the full
[128, 1000] float32 output.
"""
import sys
if '/opt/trn_rl_repo' not in sys.path:
    sys.path.insert(0, '/opt/trn_rl_repo')

import os

import numpy as np

import concourse.bass as bass
import concourse.mybir as mybir
import concourse.tile as tile
from concourse import bacc
from concourse.bass import AP
from concourse.bass_utils import run_bass_kernel_spmd

F32 = mybir.dt.float32
F32R = mybir.dt.float32r
BF16 = mybir.dt.bfloat16
RELU = mybir.ActivationFunctionType.Relu
LN = mybir.ActivationFunctionType.Ln
EXP = mybir.ActivationFunctionType.Exp
MULT = mybir.AluOpType.mult
ADD = mybir.AluOpType.add

N_CORES = 8
BPC = int(os.environ.get("ALEXNET_BPC", "16"))   # images per core
NOCC = bool(os.environ.get("ALEXNET_NOCC"))      # collectives -> local DMA (sim only)
STAGES = int(os.environ.get("ALEXNET_STAGES", "6"))
GB = N_CORES * BPC                               # global batch
NCLASS = 1000
CPS = NCLASS // N_CORES  # 125 classes per core
CPSP = 128               # padded FC3 slice width

_compiled = None  # cached nc across kernel() calls


def _patch_act_tables():
    """Make ln/exp resolve to the combined natural_log_exp_and_others set
    so the act-table-load pass emits one load instead of alternating
    between the ln-only and exp-only sets. The emitted set id is the real
    act_info.json index of the combined set, which genuinely contains
    both functions, so hardware behaviour is unchanged."""
    import concourse.bacc as bacc_mod
    if getattr(bacc_mod, '_alexnet_act_patch', None):
        return
    orig = bacc_mod.get_activation_tables

    def patched(arch):
        t = orig(arch)
        out = {}
        for name, funcs in t.items():
            if name != 'natural_log_exp_and_others' and (LN in funcs or EXP in funcs):
                funcs = funcs - {LN, EXP}
            out[name] = funcs
        return out

    bacc_mod.get_activation_tables = patched
    bacc_mod._alexnet_act_patch = True


def _lrn_chunks(nc, psp, bands, sqs, cob, xflat, out_dst, nf, t1, t2, two):
    """LRN for one <=128-channel block: banded matmul window-sum on the PE,
    then d^-0.75 = exp(-0.75*ln(2+1e-4*div)) with Ln chunked from PSUM into
    t1 (full-width f32) and a single whole-row Exp -> t2 (bf16)."""
    nb = len(sqs)
    C = xflat.shape[0]
    c0 = 0
    while c0 < nf:
        nch = min(512, nf - c0)
        psd = psp.tile([C, 512], F32, name="psd", tag="psd")
        for b in range(nb):
            lhsT = bands[b] if nb == 1 else bands[b][:, cob, :]
            nc.tensor.matmul(psd[:, :nch], lhsT, sqs[b][:, c0:c0 + nch],
                             start=(b == 0), stop=(b == nb - 1))
        nc.scalar.activation(t1[:, c0:c0 + nch], psd[:, :nch], LN,
                             bias=two[:C, 0:1], scale=1e-4)
        c0 += nch
    nc.scalar.activation(t2[:, :nf], t1[:, :nf], EXP, bias=0.0, scale=-0.75)
    nc.vector.tensor_mul(out_dst[:, :nf], xflat[:, :nf], t2[:, :nf])


def build():
    _patch_act_tables()
    nc = bacc.Bacc("TRN2", num_devices=N_CORES)

    XP = nc.dram_tensor("XP", [BPC, 3, 122, 3025], BF16, kind="ExternalInput")
    W1P = nc.dram_tensor("W1P", [3, 122, 96], BF16, kind="ExternalInput")
    W2P = nc.dram_tensor("W2P", [96, 25, 256], BF16, kind="ExternalInput")
    W3P = nc.dram_tensor("W3P", [2, 128, 9, 384], BF16, kind="ExternalInput")
    W4P = nc.dram_tensor("W4P", [3, 128, 9, 384], BF16, kind="ExternalInput")
    W5P = nc.dram_tensor("W5P", [3, 128, 9, 256], BF16, kind="ExternalInput")
    BD1 = nc.dram_tensor("BD1", [96, 96], BF16, kind="ExternalInput")
    BD2 = nc.dram_tensor("BD2", [128, 2, 2, 128], BF16, kind="ExternalInput")
    B2 = nc.dram_tensor("B2", [256], F32, kind="ExternalInput")
    B3 = nc.dram_tensor("B3", [384], F32, kind="ExternalInput")
    B4 = nc.dram_tensor("B4", [384], F32, kind="ExternalInput")
    B5 = nc.dram_tensor("B5", [256], F32, kind="ExternalInput")
    WF1 = nc.dram_tensor("WF1", [9216, 512], BF16, kind="ExternalInput")
    WF2 = nc.dram_tensor("WF2", [4096, 512], BF16, kind="ExternalInput")
    WF3 = nc.dram_tensor("WF3", [4096, CPSP], BF16, kind="ExternalInput")
    B2B = nc.dram_tensor("B2B", [256], BF16, kind="ExternalInput")
    B3B = nc.dram_tensor("B3B", [384], BF16, kind="ExternalInput")
    B4B = nc.dram_tensor("B4B", [384], BF16, kind="ExternalInput")
    B5B = nc.dram_tensor("B5B", [256], BF16, kind="ExternalInput")
    BF1B = nc.dram_tensor("BF1B", [512], BF16, kind="ExternalInput")
    BF2B = nc.dram_tensor("BF2B", [512], BF16, kind="ExternalInput")
    BF3B = nc.dram_tensor("BF3B", [CPSP], BF16, kind="ExternalInput")

    OUT = nc.dram_tensor("OUT", [CPSP, GB], F32, kind="ExternalOutput")

    with tile.TileContext(nc) as tc:
        with tc.tile_pool(name="dram", bufs=1, space="DRAM") as dpool:
            HL = dpool.tile([9216, BPC], BF16, name="HL")
            F1L = dpool.tile([512, GB], BF16, name="F1L")
            F2L = dpool.tile([512, GB], BF16, name="F2L")
            HF = dpool.tile([N_CORES * 9216 * BPC], BF16,
                            addr_space="Shared", name="HF")
            F1F = dpool.tile([4096, GB], BF16, addr_space="Shared", name="F1F")
            F2F = dpool.tile([4096, GB], BF16, addr_space="Shared", name="F2F")
            with nc.allow_low_precision(reason="bf16 activations; PSUM stays fp32"):
                _build_body(nc, tc, locals())
    nc.finalize()
    return nc


def _build_body(nc, tc, T):
    XP, W1P, W2P, W3P, W4P, W5P = T['XP'], T['W1P'], T['W2P'], T['W3P'], T['W4P'], T['W5P']
    BD1, BD2 = T['BD1'], T['BD2']
    WF1, WF2, WF3 = T['WF1'], T['WF2'], T['WF3']
    OUT = T['OUT']
    HL, F1L, F2L = T['HL'], T['F1L'], T['F2L']
    HF, F1F, F2F = T['HF'], T['F1F'], T['F2F']

    with tc.tile_pool(name="p_top", bufs=1) as p_top:
        ones_sb = p_top.tile([1, 512], BF16, name="ones_sb")
        nc.vector.memset(ones_sb[:], 1.0)
        brow = {}
        for nm, t, w in (("b2", T['B2B'], 256), ("b3", T['B3B'], 384),
                         ("b4", T['B4B'], 384), ("b5", T['B5B'], 256),
                         ("bf1", T['BF1B'], 512), ("bf2", T['BF2B'], 512),
                         ("bf3", T['BF3B'], CPSP)):
            brow[nm] = p_top.tile([1, w], BF16, name=f"brow_{nm}")
            nc.sync.dma_start(brow[nm][:], t.ap().unsqueeze(0))
        _build_inner(nc, tc, T, ones_sb, brow)


def _build_inner(nc, tc, T, ones_sb, brow):
    XP, W1P, W2P, W3P, W4P, W5P = T['XP'], T['W1P'], T['W2P'], T['W3P'], T['W4P'], T['W5P']
    BD1, BD2 = T['BD1'], T['BD2']
    WF1, WF2, WF3 = T['WF1'], T['WF2'], T['WF3']
    OUT = T['OUT']
    HL, F1L, F2L = T['HL'], T['F1L'], T['F2L']
    HF, F1F, F2F = T['HF'], T['F1F'], T['F2F']
    with tc.tile_pool(name="p_c3in", bufs=1) as p_c3in:
        # conv3 input, padded, SBUF-resident: 2 channel blocks (flat +4 pad)
        c3in = [p_c3in.tile([128, BPC * 225 + 4], BF16, name=f"c3in{b}")
                for b in range(2)]
        c3in_v = [t[:, :BPC * 225].rearrange("p (i a b) -> p i a b",
                                             i=BPC, a=15) for t in c3in]
        nc.gpsimd.memset(c3in[0][:], 0.0)
        nc.gpsimd.memset(c3in[1][:], 0.0)

        with tc.tile_pool(name="p_ab", bufs=1) as p_ab:
            w1_sb = p_ab.tile([122, 3, 96], BF16, name="w1_sb")
            nc.sync.dma_start(w1_sb[:],
                              AP(W1P, 0, [[96, 122], [122 * 96, 3], [1, 96]]))
            bd1_sb = p_ab.tile([96, 96], BF16, name="bd1_sb")
            nc.sync.dma_start(bd1_sb[:], BD1[:])
            two_sb = p_ab.tile([128, 1], F32, name="two_sb")
            nc.vector.memset(two_sb[:], 2.0)
            # conv2 input, padded, SBUF-resident
            c2in = p_ab.tile([96, BPC, 31, 31], BF16, name="c2in")
            nc.gpsimd.memset(c2in[:], 0.0)

            # ======== stage A: conv1 + relu + LRN + pool ========
            with tc.tile_pool(name="p_a", bufs=1) as p_a, \
                 tc.tile_pool(name="ps_a", bufs=4, space="PSUM") as ps_a, \
                 tc.tile_pool(name="ps_al", bufs=2, space="PSUM") as ps_al:
                for img in range(BPC):
                    # partition p = ky*11 + kx (121 taps; row 121 = ones for
                    # the bias); value at (ci, y*55+x) = padded[ci, 4y+ky, 4x+kx]
                    c1in = p_a.tile([122, 3, 3025], BF16, name="c1in",
                                    tag="c1in", bufs=2)
                    nc.sync.dma_start(
                        c1in[:],
                        AP(XP.tensor, img * 3 * 122 * 3025,
                           [[3025, 122], [122 * 3025, 3], [1, 3025]]))
                    c1o = p_a.tile([96, 3025], BF16, name="c1o", tag="c1o", bufs=3)
                    c0 = 0
                    while c0 < 3025:
                        nch = min(512, 3025 - c0)
                        ps = ps_a.tile([96, 512], F32, name="c1ps", tag="c1ps")
                        for ci in range(3):
                            nc.tensor.matmul(ps[:, :nch], w1_sb[:, ci, :],
                                             c1in[:, ci, c0:c0 + nch],
                                             start=(ci == 0), stop=(ci == 2))
                        nc.vector.tensor_scalar_max(c1o[:, c0:c0 + nch],
                                                    ps[:, :nch], 0.0)
                        c0 += nch
                    # LRN over the whole image (banded matmul for window sum)
                    nf = 3025
                    sq = p_a.tile([96, 3025], BF16, name="sq_a", tag="sq_a",
                                  bufs=3)
                    xl = p_a.tile([96, 3025], BF16, name="xl_a", tag="xl_a",
                                  bufs=3)
                    t1 = p_a.tile([96, 3025], F32, name="t1_a", tag="t1_a",
                                  bufs=2)
                    t2 = p_a.tile([96, 3025], BF16, name="t2_a", tag="t2_a",
                                  bufs=2)
                    xf = c1o[:]
                    nc.vector.tensor_mul(sq[:], xf, xf)
                    _lrn_chunks(nc, ps_al, [bd1_sb[:]], [sq], 0, xf,
                                xl, nf, t1, t2, two_sb)
                    xl3 = xl[:].rearrange("p (a b) -> p a b", a=55)
                    # pool 3x3 s2 -> [96, 27, 27] into c2in interior
                    htmp = p_a.tile([96, 55, 27], BF16, name="htmp", tag="htmp", bufs=3)
                    nc.vector.tensor_max(htmp[:], xl3[:, :, 0:53:2],
                                         xl3[:, :, 1:54:2])
                    nc.vector.tensor_max(htmp[:], htmp[:], xl3[:, :, 2:55:2])
                    dst = c2in[:, img, 2:29, 2:29]
                    nc.vector.tensor_max(dst, htmp[:, 0:53:2, :],
                                         htmp[:, 1:54:2, :])
                    nc.vector.tensor_max(dst, dst, htmp[:, 2:55:2, :])

            if STAGES < 2:
                return
            # ======== stage B: conv2 + relu + LRN + pool ========
            with tc.tile_pool(name="p_b", bufs=1) as p_b, \
                 tc.tile_pool(name="ps_b", bufs=4, space="PSUM") as ps_b, \
                 tc.tile_pool(name="ps_bl", bufs=2, space="PSUM") as ps_bl:
                w2_sb = p_b.tile([96, 25, 256], BF16, name="w2_sb")
                nc.sync.dma_start(w2_sb[:], W2P[:])
                bd2_sb = p_b.tile([128, 2, 2, 128], BF16, name="bd2_sb")
                nc.sync.dma_start(bd2_sb[:], BD2[:])
                for img in range(BPC):
                    c2o = [None, None]
                    sq = [None, None]
                    for cb in range(2):
                        c2o[cb] = p_b.tile([128, 27, 27], BF16, name=f"c2o{cb}",
                                           tag=f"c2o{cb}", bufs=2)
                        for (yy0, rows) in ((0, 14), (14, 13)):
                            nn = rows * 27
                            ps = ps_b.tile([128, 14 * 27], F32, name="c2ps",
                                           tag="c2ps")
                            nc.tensor.matmul(
                                ps[:, :nn],
                                brow["b2"][:, cb * 128:(cb + 1) * 128],
                                ones_sb[:, :nn],
                                start=True, stop=False)
                            for o in range(25):
                                ky, kx = divmod(o, 5)
                                nc.tensor.matmul(
                                    ps[:, :nn],
                                    w2_sb[:, o, cb * 128:(cb + 1) * 128],
                                    c2in[:, img, yy0 + ky:yy0 + ky + rows,
                                         kx:kx + 27],
                                    start=False, stop=(o == 24))
                            nc.vector.tensor_scalar_max(
                                c2o[cb][:, yy0:yy0 + rows, :].rearrange(
                                    "p a b -> p (a b)"),
                                ps[:, :nn], 0.0)
                        sq[cb] = p_b.tile([128, 729], BF16, name=f"sqb{cb}",
                                          tag=f"sqb{cb}", bufs=2)
                        xfc = c2o[cb][:].rearrange("p a b -> p (a b)")
                        nc.vector.tensor_mul(sq[cb][:], xfc, xfc)
                    for cb in range(2):
                        xl = p_b.tile([128, 729], BF16, name="xlb", tag="xlb",
                                      bufs=2)
                        t1 = p_b.tile([128, 729], F32, name="t1_b", tag="t1_b",
                                      bufs=2)
                        t2 = p_b.tile([128, 729], BF16, name="t2_b", tag="t2_b",
                                      bufs=2)
                        xf = c2o[cb][:].rearrange("p a b -> p (a b)")
                        _lrn_chunks(nc, ps_bl,
                                    [bd2_sb[:, 0], bd2_sb[:, 1]],
                                    sq, cb, xf, xl[:], 729, t1, t2, two_sb)
                        # pool 27 -> 13 into c3in interior
                        xl3 = xl[:].rearrange("p (a b) -> p a b", a=27)
                        h2 = p_b.tile([128, 27, 13], BF16, name="htmp2", tag="htmp2", bufs=2)
                        nc.vector.tensor_max(h2[:], xl3[:, :, 0:25:2],
                                             xl3[:, :, 1:26:2])
                        nc.vector.tensor_max(h2[:], h2[:], xl3[:, :, 2:27:2])
                        dst = c3in_v[cb][:, img, 1:14, 1:14]
                        nc.vector.tensor_max(dst, h2[:, 0:25:2, :], h2[:, 1:26:2, :])
                        nc.vector.tensor_max(dst, dst, h2[:, 2:27:2, :])

        if STAGES < 3:
            return
        with tc.tile_pool(name="p_fcw", bufs=1) as p_fcw:
            # prefetch all FC weights; overlaps conv3-5
            wf1_sb = p_fcw.tile([128, 72, 512], BF16, name="wf1_sb")
            nc.sync.dma_start(wf1_sb[:],
                              AP(WF1, 0, [[512, 128], [128 * 512, 72], [1, 512]]))
            wf2_sb = p_fcw.tile([128, 32, 512], BF16, name="wf2_sb")
            nc.sync.dma_start(wf2_sb[:],
                              AP(WF2, 0, [[512, 128], [128 * 512, 32], [1, 512]]))
            wf3_sb = p_fcw.tile([128, 32, CPSP], BF16, name="wf3_sb")
            nc.sync.dma_start(wf3_sb[:],
                              AP(WF3, 0, [[CPSP, 128], [128 * CPSP, 32], [1, CPSP]]))
            with tc.tile_pool(name="p_45", bufs=1) as p_45:
                # conv4/conv5 inputs, padded, SBUF-resident (3 channel blocks)
                c4in = [p_45.tile([128, BPC * 225 + 4], BF16, name=f"c4in{b}")
                        for b in range(3)]
                c4in_v = [t[:, :BPC * 225].rearrange("p (i a b) -> p i a b",
                                                     i=BPC, a=15) for t in c4in]
                c5in = [p_45.tile([128, BPC * 225 + 4], BF16, name=f"c5in{b}")
                        for b in range(3)]
                c5in_v = [t[:, :BPC * 225].rearrange("p (i a b) -> p i a b",
                                                     i=BPC, a=15) for t in c5in]
                for b in range(3):
                    nc.gpsimd.memset(c4in[b][:], 0.0)
                    nc.gpsimd.memset(c5in[b][:], 0.0)

                # ======== stage C: conv3 + relu -> c4in (SBUF) ========
                with tc.tile_pool(name="p_c", bufs=1) as p_c, \
                     tc.tile_pool(name="ps_c", bufs=6, space="PSUM") as ps_c:
                    w3_sb = [p_c.tile([128, 9, 384], BF16, name=f"w3_{cib}")
                             for cib in range(2)]
                    for cib in range(2):
                        nc.sync.dma_start(w3_sb[cib][:], W3P[cib])
                    for p in range(BPC // 2):
                        for cob in range(3):
                            ps = ps_c.tile([128, 452], F32, name="c3ps",
                                           tag="c3ps")
                            nc.tensor.matmul(
                                ps[:, :422],
                                brow["b3"][:, cob * 128:(cob + 1) * 128],
                                ones_sb[:, :422], start=True, stop=False)
                            for cib in range(2):
                                for o in range(9):
                                    ky, kx = divmod(o, 3)
                                    off = 2 * p * 225 + ky * 15 + kx
                                    nc.tensor.matmul(
                                        ps[:, :422],
                                        w3_sb[cib][:, o,
                                                   cob * 128:(cob + 1) * 128],
                                        c3in[cib][:, off:off + 422],
                                        start=False,
                                        stop=(cib == 1 and o == 8))
                            psv = ps[:, :450].rearrange(
                                "p (i a b) -> p i a b",
                                i=2, a=15)[:, :, 0:13, 0:13]
                            nc.vector.tensor_scalar_max(
                                c4in_v[cob][:, 2 * p:2 * p + 2, 1:14, 1:14],
                                psv, 0.0)

                if STAGES < 4:
                    return
                # ======== stage D: conv4 + relu -> c5in (SBUF) ========
                with tc.tile_pool(name="p_d", bufs=1) as p_d, \
                     tc.tile_pool(name="ps_d", bufs=6, space="PSUM") as ps_d:
                    w4_sb = [p_d.tile([128, 9, 384], BF16, name=f"w4_{cib}")
                             for cib in range(3)]
                    for cib in range(3):
                        nc.sync.dma_start(w4_sb[cib][:], W4P[cib])
                    for p in range(BPC // 2):
                        for cob in range(3):
                            ps = ps_d.tile([128, 452], F32, name="c4ps",
                                           tag="c4ps")
                            nc.tensor.matmul(
                                ps[:, :422],
                                brow["b4"][:, cob * 128:(cob + 1) * 128],
                                ones_sb[:, :422], start=True, stop=False)
                            for cib in range(3):
                                for o in range(9):
                                    ky, kx = divmod(o, 3)
                                    off = 2 * p * 225 + ky * 15 + kx
                                    nc.tensor.matmul(
                                        ps[:, :422],
                                        w4_sb[cib][:, o,
                                                   cob * 128:(cob + 1) * 128],
                                        c4in[cib][:, off:off + 422],
                                        start=False,
                                        stop=(cib == 2 and o == 8))
                            psv = ps[:, :450].rearrange(
                                "p (i a b) -> p i a b",
                                i=2, a=15)[:, :, 0:13, 0:13]
                            nc.vector.tensor_scalar_max(
                                c5in_v[cob][:, 2 * p:2 * p + 2, 1:14, 1:14],
                                psv, 0.0)

                if STAGES < 5:
                    return
                # ======== stage E: conv5 + relu + pool ========
                with tc.tile_pool(name="p_e", bufs=1) as p_e, \
                     tc.tile_pool(name="ps_e", bufs=6, space="PSUM") as ps_e:
                    w5_sb = [p_e.tile([128, 9, 256], BF16, name=f"w5_{cib}")
                             for cib in range(3)]
                    for cib in range(3):
                        nc.sync.dma_start(w5_sb[cib][:], W5P[cib])
                    # hl layout: [feat_p, y, x, img] -- img innermost so the
                    # HL store is contiguous (feature-major, image runs)
                    hl_sb = [p_e.tile([128, 6, 6, BPC], BF16, name=f"hl{cob}")
                             for cob in range(2)]
                    for p in range(BPC // 2):
                        for cob in range(2):
                            ps = ps_e.tile([128, 452], F32, name="c5ps",
                                           tag="c5ps")
                            nc.tensor.matmul(
                                ps[:, :422],
                                brow["b5"][:, cob * 128:(cob + 1) * 128],
                                ones_sb[:, :422], start=True, stop=False)
                            for cib in range(3):
                                for o in range(9):
                                    ky, kx = divmod(o, 3)
                                    off = 2 * p * 225 + ky * 15 + kx
                                    nc.tensor.matmul(
                                        ps[:, :422],
                                        w5_sb[cib][:, o,
                                                   cob * 128:(cob + 1) * 128],
                                        c5in[cib][:, off:off + 422],
                                        start=False,
                                        stop=(cib == 2 and o == 8))
                            c5o = p_e.tile([128, 2, 13, 13], BF16, name="c5o",
                                           tag="c5o", bufs=3)
                            psv = ps[:, :450].rearrange(
                                "p (i a b) -> p i a b",
                                i=2, a=15)[:, :, 0:13, 0:13]
                            nc.vector.tensor_scalar_max(c5o[:], psv, 0.0)
                            # maxpool 13 -> 6
                            vt = p_e.tile([128, 2, 6, 13], BF16, name="vt",
                                          tag="vt")
                            nc.vector.tensor_max(vt[:], c5o[:, :, 0:11:2, :],
                                                 c5o[:, :, 1:12:2, :])
                            nc.vector.tensor_max(vt[:], vt[:],
                                                 c5o[:, :, 2:13:2, :])
                            dst = hl_sb[cob][:, :, :, 2 * p:2 * p + 2] \
                                .rearrange("p a b i -> p i a b")
                            nc.vector.tensor_max(dst, vt[:, :, :, 0:11:2],
                                                 vt[:, :, :, 1:12:2])
                            nc.vector.tensor_max(dst, dst, vt[:, :, :, 2:13:2])
                    # write HL [9216, BPC]: row = c_global*36 + (y*6+x),
                    # col = img -- contiguous per cob block
                    for cob in range(2):
                        dst = AP(HL.tensor, cob * 128 * 36 * BPC,
                                 [[36 * BPC, 128], [1, 36 * BPC]])
                        nc.sync.dma_start(
                            dst, hl_sb[cob][:].rearrange("p a b i -> p (a b i)"))

            if STAGES < 6:
                return
            _build_fc(nc, tc, T, ones_sb, brow, wf1_sb, wf2_sb, wf3_sb)


def _build_fc(nc, tc, T, ones_sb, brow, wf1_sb, wf2_sb, wf3_sb):
    OUT = T['OUT']
    HL, F1L, F2L = T['HL'], T['F1L'], T['F2L']
    HF, F1F, F2F = T['HF'], T['F1F'], T['F2F']
    # ======== FC stages (feature-major: weights as lhsT, acts as rhs) ====
    if NOCC:
        nc.gpsimd.dma_start(HF[:9216 * BPC], HL[:].rearrange("a b -> (a b)"))
    else:
        nc.gpsimd.collective_compute(
            "AllGather", mybir.AluOpType.bypass,
            replica_groups=[list(range(N_CORES))],
            ins=[HL[:].rearrange("a b -> (a b)").opt()], outs=[HF[:].opt()])

    with tc.tile_pool(name="p_f", bufs=1) as p_f, \
         tc.tile_pool(name="ps_f", bufs=2, space="PSUM") as ps_f:
        # all FC right-hand activations live in SBUF, feature-major
        h_sb = p_f.tile([128, 72, N_CORES, BPC], BF16, name="h_sb")
        for c in range(N_CORES):
            src = AP(HF.tensor, c * 9216 * BPC,
                     [[BPC, 128], [128 * BPC, 72], [1, BPC]])
            nc.sync.dma_start(h_sb[:, :, c, :], src)

        # FC1: psf1[fo, img] = Wf1[fo, :] @ h
        f1o = p_f.tile([128, 4, GB], BF16, name="f1o")
        for b in range(4):
            ps = ps_f.tile([128, GB], F32, name="psf1", tag="psf")
            nc.tensor.matmul(ps[:], brow["bf1"][:, b * 128:(b + 1) * 128],
                             ones_sb[:, :GB], start=True, stop=False)
            for j in range(72):
                nc.tensor.matmul(ps[:],
                                 wf1_sb[:, j, b * 128:(b + 1) * 128],
                                 h_sb[:, j].rearrange("p a b -> p (a b)"),
                                 start=False, stop=(j == 71))
            nc.vector.tensor_scalar_max(f1o[:, b, :], ps[:], 0.0)
        nc.sync.dma_start(AP(F1L.tensor, 0, [[GB, 128], [128 * GB, 4], [1, GB]]),
                          f1o[:])
        if NOCC:
            nc.gpsimd.dma_start(F1F[0:512, :], F1L[:])
        else:
            nc.gpsimd.collective_compute(
                "AllGather", mybir.AluOpType.bypass,
                replica_groups=[list(range(N_CORES))],
                ins=[F1L[:].rearrange("a b -> (a b)").opt()],
                outs=[F1F[:].rearrange("a b -> (a b)").opt()])

        # FC2
        f1f_sb = p_f.tile([128, 32, GB], BF16, name="f1f_sb")
        src = AP(F1F.tensor, 0, [[GB, 128], [128 * GB, 32], [1, GB]])
        nc.sync.dma_start(f1f_sb[:], src)
        f2o = p_f.tile([128, 4, GB], BF16, name="f2o")
        for b in range(4):
            ps = ps_f.tile([128, GB], F32, name="psf2", tag="psf")
            nc.tensor.matmul(ps[:], brow["bf2"][:, b * 128:(b + 1) * 128],
                             ones_sb[:, :GB], start=True, stop=False)
            for j in range(32):
                nc.tensor.matmul(ps[:],
                                 wf2_sb[:, j, b * 128:(b + 1) * 128],
                                 f1f_sb[:, j], start=False, stop=(j == 31))
            nc.vector.tensor_scalar_max(f2o[:, b, :], ps[:], 0.0)
        nc.sync.dma_start(AP(F2L.tensor, 0, [[GB, 128], [128 * GB, 4], [1, GB]]),
                          f2o[:])
        if NOCC:
            nc.gpsimd.dma_start(F2F[0:512, :], F2L[:])
        else:
            nc.gpsimd.collective_compute(
                "AllGather", mybir.AluOpType.bypass,
                replica_groups=[list(range(N_CORES))],
                ins=[F2L[:].rearrange("a b -> (a b)").opt()],
                outs=[F2F[:].rearrange("a b -> (a b)").opt()])

        # FC3
        f2f_sb = p_f.tile([128, 32, GB], BF16, name="f2f_sb")
        src = AP(F2F.tensor, 0, [[GB, 128], [128 * GB, 32], [1, GB]])
        nc.sync.dma_start(f2f_sb[:], src)
        psf3 = ps_f.tile([CPSP, GB], F32, name="psf3", tag="psf")
        nc.tensor.matmul(psf3[:], brow["bf3"][:, :CPSP],
                         ones_sb[:, :GB], start=True, stop=False)
        for j in range(32):
            nc.tensor.matmul(psf3[:], wf3_sb[:, j, :], f2f_sb[:, j],
                             start=False, stop=(j == 31))
        oo = p_f.tile([CPSP, GB], F32, name="oo")
        nc.vector.tensor_scalar_max(oo[:], psf3[:], 0.0)
        nc.sync.dma_start(OUT[:], oo[:])


def _band(n):
    m = np.zeros((n, n), np.float32)
    for i in range(n):
        m[max(0, i - 2):i + 3, i] = 1.0
    return m


def _prep_inputs(x, W1, b1, W2, b2, W3, b3, W4, b4, W5, b5,
                 Wf1, bf1, Wf2, bf2, Wf3, bf3):
    import ml_dtypes
    bf = ml_dtypes.bfloat16
    f = np.float32
    from numpy.lib.stride_tricks import sliding_window_view
    xpad = np.pad(np.asarray(x, f), ((0, 0), (0, 0), (2, 2), (2, 2))).astype(bf)
    B = xpad.shape[0]
    # conv1 input layout: [B, ci, p=(ky*11+kx), y*55+x] = padded[ci, 4y+ky, 4x+kx]
    sw = sliding_window_view(xpad, (11, 11), axis=(2, 3))[:, :, ::4, ::4]
    xp = np.empty((B, 3, 122, 3025), bf)
    xp[:, :, :121, :] = sw.transpose(0, 1, 4, 5, 2, 3).reshape(B, 3, 121, 3025)
    xp[:, :, 121, :] = bf(1.0)
    # conv1 weights: [ci, p=(ky*11+kx), co]; row 121 of ci=0 carries the bias
    W1p = np.zeros((3, 122, 96), f)
    W1p[:, :121, :] = np.asarray(W1, f).transpose(1, 2, 3, 0).reshape(3, 121, 96)
    W1p[0, 121, :] = np.asarray(b1, f)
    W1p = W1p.astype(bf)
    W2p = np.ascontiguousarray(
        np.asarray(W2, f).transpose(1, 2, 3, 0).reshape(96, 25, 256)).astype(bf)
    W3p = np.ascontiguousarray(
        np.asarray(W3, f).transpose(1, 2, 3, 0).reshape(2, 128, 9, 384)).astype(bf)
    W4p = np.ascontiguousarray(
        np.asarray(W4, f).transpose(1, 2, 3, 0).reshape(3, 128, 9, 384)).astype(bf)
    W5p = np.ascontiguousarray(
        np.asarray(W5, f).transpose(1, 2, 3, 0).reshape(3, 128, 9, 256)).astype(bf)
    # BD2[i, cib, cob, j] = 1 iff |cib*128+i - (cob*128+j)| <= 2
    bd2 = np.zeros((128, 2, 2, 128), np.float32)
    for cib in range(2):
        for cob in range(2):
            for i in range(128):
                lo = max(cib * 128 + i - 2 - cob * 128, 0)
                hi = min(cib * 128 + i + 2 - cob * 128, 127)
                if lo <= hi:
                    bd2[i, cib, cob, lo:hi + 1] = 1.0
    in_maps = []
    for c in range(N_CORES):
        cs, ce = c * 512, (c + 1) * 512
        ks, ke = c * CPS, (c + 1) * CPS
        m = dict(
            XP=np.ascontiguousarray(xp[c * BPC:(c + 1) * BPC]),
            W1P=W1p, W2P=W2p, W3P=W3p, W4P=W4p, W5P=W5p,
            BD1=_band(96).astype(bf), BD2=bd2.astype(bf),
            B2=np.asarray(b2, f), B3=np.asarray(b3, f),
            B4=np.asarray(b4, f), B5=np.asarray(b5, f),
            B2B=np.asarray(b2, f).astype(bf),
            B3B=np.asarray(b3, f).astype(bf),
            B4B=np.asarray(b4, f).astype(bf),
            B5B=np.asarray(b5, f).astype(bf),
            BF1B=np.asarray(bf1, f)[cs:ce].astype(bf),
            BF2B=np.asarray(bf2, f)[cs:ce].astype(bf),
            BF3B=np.pad(np.asarray(bf3, f)[ks:ke], (0, 3)).astype(bf),
            WF1=np.ascontiguousarray(np.asarray(Wf1, f)[cs:ce].T).astype(bf),
            WF2=np.ascontiguousarray(np.asarray(Wf2, f)[cs:ce].T).astype(bf),
            WF3=np.ascontiguousarray(
                np.pad(np.asarray(Wf3, f)[ks:ke], ((0, 3), (0, 0))).T).astype(bf),
        )
        in_maps.append(m)
    return in_maps


def _get_nc():
    global _compiled
    if _compiled is None:
        _compiled = build()
    return _compiled


def kernel(**inputs):
    nc = _get_nc()
    in_maps = _prep_inputs(**inputs)
    res = run_bass_kernel_spmd(nc, in_maps, list(range(N_CORES)))
    return np.concatenate(
        [res.results[c]["OUT"][:CPS, :].T for c in range(N_CORES)],
        axis=1).astype(np.float32)


def run_traced(**inputs):
    """Like kernel() but with NTFF tracing; returns (output, BassKernelResults)."""
    nc = _get_nc()
    in_maps = _prep_inputs(**inputs)
    res = run_bass_kernel_spmd(nc, in_maps, list(range(N_CORES)), trace=True)
    out = np.concatenate(
        [res.results[c]["OUT"][:CPS, :].T for c in range(N_CORES)],
        axis=1).astype(np.float32)
    return out, res
